# revision 32
# baseline (speedup 1.0000x reference)
"""Trainium2 Bass kernel for nn_Attention (B=4, N=2048, C=768, H=12).

Sharding: 8 cores = 4 batches x 2 head-groups (6 heads each), Megatron-style
tensor parallel on the heads. Each core computes qkv for its head slice,
attention for 6 heads, and per-head-pair output-projection partials
out3 [3, 2048, 768]. The host sums the 3 pair partials of the 2 cores
covering each batch and adds the bias.

Per-core attention scheme (no transposes anywhere):
  - q,k stored [d, n] (feature-major) straight out of the QKV matmul; heads
    packed in pairs per 128-partition group (head 2p -> partitions 0-63,
    head 2p+1 -> 64-127).
  - S^T tiles [128 j, i] = k_chunk.T @ q  (K=64 matmul). exp() on scalar
    engine reads PSUM, writes SBUF. No max subtraction (logits are O(10);
    softmax is shift-invariant so this only perturbs rounding).
  - v stored [n, d] with an extra ones column; PV matmul lhsT=v[j,0:65],
    rhs=exp(S^T) accumulates [65, 512] where row 64 = sum_j exp = Z.
  - normalize: 1/Z broadcast across partitions via a DRAM-bounce DMA, one
    DVE multiply; odd heads' results are DMA'd up to partitions 64-127
    (engines cannot shift partitions; DMA can).

Matmuls default to float32r (TF32-class, 4x faster than fp32 on the PE;
measured 3.8e-4 scale-relative absmax error vs the fp32 reference).
Set KERNEL_MM_DT=float32 for full fp32 precision (3.4e-6) at ~3x the time.
"""

import os
import sys
from contextlib import ExitStack

if "/opt/trn_rl_repo" not in sys.path:
    sys.path.insert(0, "/opt/trn_rl_repo")

import numpy as np

import concourse.bass as bass
import concourse.mybir as mybir
import concourse.tile as tile
from concourse import bass_utils

F32 = mybir.dt.float32

B, N, C = 4, 2048, 768
NH, D = 12, 64
SCALE = D ** -0.5
HPC = NH // 2          # heads per core
F = HPC * D            # 384 per-core features per projection
QKVF = 3 * F           # 1152
P = 128
CO = C // P            # 6 contraction chunks
FO = F // P            # 3 feature chunks (head pairs)
NO = N // P            # 16 token chunks of 128
NCORES = 8

_MM_DT_NAME = os.environ.get("KERNEL_MM_DT", "float32r")
MM_DT = getattr(mybir.dt, _MM_DT_NAME)


def _d(ap):
    """Cast an fp32 AP to the matmul compute dtype (bitcast, same bytes)."""
    return ap.bitcast(MM_DT) if MM_DT != F32 else ap


def _r(ap):
    """Cast a producer OUT AP feeding a matmul to the compute dtype, so the
    producing engine rounds to fp32r (walrus verifies this chain)."""
    return ap.bitcast(MM_DT) if MM_DT == mybir.dt.float32r else ap


def _split_multiwaits(nc):
    """This container's walrus accepts at most ONE sync-wait per instruction.

    Split any instruction carrying N>1 waits into (N-1) single-wait NOPs on
    the same engine queue placed immediately before it (engine queues are
    FIFO, so the semantics are identical)."""
    ctr = 0
    for f in nc.m.functions:
        for blk in f.blocks:
            insts = blk.instructions
            out = []
            changed = False
            for ins in insts:
                si = ins.sync_info
                if si is not None and len(si.on_wait) > 1:
                    changed = True
                    waits = list(si.on_wait)
                    for ww in waits[:-1]:
                        nop = mybir.InstNoOp(name=f"zzsplitw_{ctr}", ins=[], outs=[])
                        ctr += 1
                        nop.engine = ins.engine
                        nop.sync_info = mybir.SyncInfo(on_wait=[ww], on_update=[])
                        out.append(nop)
                    ins.sync_info = mybir.SyncInfo(
                        on_wait=[waits[-1]], on_update=list(si.on_update)
                    )
                out.append(ins)
            if changed:
                blk.instructions = out
    return nc


def _emit(nc, tc, ctx):
    xT = nc.dram_tensor("xT", [C, N], F32, kind="ExternalInput").ap()
    wqkvT = nc.dram_tensor("wqkvT", [C, QKVF], F32, kind="ExternalInput").ap()
    wprojT = nc.dram_tensor("wprojT", [F, C], F32, kind="ExternalInput").ap()
    out3 = nc.dram_tensor("out3", [FO, N, C], F32, kind="ExternalOutput").ap()

    persist = ctx.enter_context(tc.tile_pool(name="persist", bufs=1))

    # q/k in [feature, token] layout, split per (pair, 512-token chunk) so
    # consumers wait only on the producer they actually need (Tile tracks
    # dependencies at whole-tile granularity).
    q_sb = [[persist.tile([P, 512], F32, tag=f"q{fo}_{n4}", name=f"q{fo}_{n4}")
             for n4 in range(4)] for fo in range(FO)]
    k_sb = [[persist.tile([P, 512], F32, tag=f"k{fo}_{n4}", name=f"k{fo}_{n4}")
             for n4 in range(4)] for fo in range(FO)]
    # v in [token, feature] layout per 128-token chunk, +1 ones column.
    v_sb = [persist.tile([P, HPC, D + 1], F32, tag=f"v{no}", name=f"v{no}") for no in range(NO)]
    # attention output per pair, [feature, token] layout; 2 rotating slots
    # (pair 2 reuses pair 0's slot once proj-0 has drained it)
    otp = ctx.enter_context(tc.tile_pool(name="otp", bufs=2))
    ot_sb = [otp.tile([P, N], F32, tag="ot", name=f"ot{pr}") for pr in range(FO)]
    wp_sb = persist.tile([P, FO, C], F32, tag="wp")

    ones_sb = persist.tile([P, HPC], F32, tag="ones")
    nc.vector.memset(ones_sb, 1.0)
    for no in range(NO):
        # DVE copy (not memset) so the output can be declared fp32r
        nc.vector.tensor_copy(out=_r(v_sb[no][:, :, D : D + 1]), in_=ones_sb)
    # dummy exp: pulls the ~2.7us ACT table load into the DMA lead-in window
    expwarm = persist.tile([P, HPC], F32, tag="expwarm")
    nc.scalar.activation(
        out=expwarm,
        in_=ones_sb,
        func=mybir.ActivationFunctionType.Exp,
        scale=1.0,
    )

    with (
        tc.tile_pool(name="wqp", bufs=1) as wqp,
        tc.tile_pool(name="xs", bufs=3) as xs_pool,
        tc.tile_pool(name="ptp", bufs=3) as pt_pool,
        tc.tile_pool(name="rp", bufs=2) as r_pool,
        tc.tile_pool(name="outp", bufs=1) as outp,
        tc.tile_pool(name="rd", bufs=3, space="DRAM") as rd_pool,
        tc.tile_pool(name="ps1", bufs=2, space="PSUM") as ps1,
        tc.tile_pool(name="ps_st", bufs=2, space="PSUM") as ps_st,
        tc.tile_pool(name="ps_o", bufs=2, space="PSUM") as ps_o,
    ):
        # weight tiles per (column-section, contraction chunk) so each qkv
        # matmul depends on exactly one DMA
        wq_tiles = {}

        def load_wq(slices):
            for lo, hi in slices:
                for co in range(CO):
                    t = wqp.tile([P, hi - lo], F32, tag=f"wq_{lo}_{co}",
                                 name=f"wq_{lo}_{co}")
                    wq_tiles[(lo, co)] = t
                    nc.sync.dma_start(
                        out=_r(t),
                        in_=_r(wqkvT[co * P : (co + 1) * P, lo:hi]),
                    )

        def wq_slice(foff, co, width=P):
            """AP for weight columns [foff, foff+width) of chunk co."""
            for lo, hi in ((0, P), (F, F + P), (2 * F, 3 * F), (P, F), (F + P, 2 * F)):
                if lo <= foff and foff + width <= hi:
                    return wq_tiles[(lo, co)][:, foff - lo : foff - lo + width]
            raise KeyError(foff)

        def emit_qkv_pass(fo):
            """q/k chunk fo over all tokens.

            Streams xT per 512-token slice (xT is re-read from DRAM once per
            pass; DMA is far from the bottleneck and this keeps SBUF free)."""
            xt4s = []
            for n4 in range(4):
                xt4 = xs_pool.tile([P, CO, 512], F32, tag="xt4",
                                   name=f"xt4_{fo}_{n4}")
                xt4s.append(xt4)
                # one DMA per contraction chunk so the first matmul can
                # start after ~256KB instead of the full 1.5MB slice
                for co in range(CO):
                    nc.sync.dma_start(
                        out=_r(xt4[:, co, :]),
                        in_=_r(xT[co * P : (co + 1) * P,
                                  n4 * 512 : (n4 + 1) * 512]),
                    )
                for dst, foff in ((q_sb[fo][n4], fo * P), (k_sb[fo][n4], F + fo * P)):
                    pq = ps1.tile([P, 512], F32, tag="pqk")
                    for co in range(CO):
                        nc.tensor.matmul(
                            pq,
                            _d(wq_slice(foff, co)),
                            _d(xt4[:, co, :]),
                            start=(co == 0),
                            stop=(co == CO - 1),
                        )
                    nc.vector.tensor_copy(out=_r(dst), in_=pq)
            return xt4s

        def emit_v_chunk(no, xtv):
            """v for one 128-token chunk, reading an [P, CO, 512] x-slice."""
            pv = ps1.tile([P, F], F32, tag="pqk", name=f"pv_{no}")
            for co in range(CO):
                nc.tensor.matmul(
                    pv,
                    _d(xtv[:, co, (no % 4) * P : (no % 4 + 1) * P]),
                    _d(wq_slice(2 * F, co, F)),
                    start=(co == 0),
                    stop=(co == CO - 1),
                )
            nc.vector.tensor_copy(
                out=_r(v_sb[no][:, :, 0:D]),
                in_=pv.rearrange("p (h d) -> p h d", h=HPC),
            )

        def emit_normalize(po, pr, plo, i512):
            # evacuate PSUM -> SBUF at once so the po slot frees for the next
            # i512 block (the normalize chain below has DMA latency in it)
            ov = r_pool.tile([65, 512], F32, tag="ov", name=f"ov_{pr}_{plo}_{i512}")
            nc.vector.tensor_copy(out=ov, in_=po)
            # 1/Z lives on partition 64 (engines cannot move data across
            # partitions, so compute in place on lane 64)
            nc.vector.reciprocal(out=ov[64:65, :], in_=ov[64:65, :])
            # partition-broadcast 1/Z: SBUF zero-step partition APs are
            # illegal, so bounce through DRAM (DRAM APs broadcast fine)
            rdram = rd_pool.tile([1, 512], F32, tag="rd", name=f"rd_{pr}_{plo}_{i512}")
            nc.sync.dma_start(out=rdram, in_=ov[64:65, :])
            rb = r_pool.tile([64, 512], F32, tag="rb", name=f"rb_{pr}_{plo}_{i512}")
            nc.sync.dma_start(out=rb, in_=rdram.to_broadcast([64, 512]))
            if plo == 0:
                nc.vector.tensor_mul(
                    out=_r(ot_sb[pr][0:64, i512 : i512 + 512]),
                    in0=ov[0:64, :],
                    in1=rb,
                )
            else:
                # odd head: normalize at partitions 0-63, then DMA up to
                # partitions 64-127 of ot
                nt = r_pool.tile([64, 512], F32, tag="nt", name=f"nt_{pr}_{i512}")
                nc.vector.tensor_mul(out=_r(nt), in0=ov[0:64, :], in1=rb)
                nc.sync.dma_start(
                    out=_r(ot_sb[pr][64:128, i512 : i512 + 512]), in_=_r(nt)
                )

        # interleave: qkv pass for a head pair, then that pair's attention.
        # Both heads of a pair share one [128, 1024] S^T tile (head A cols
        # 0-511, head B cols 512-1023): their K=64 matmuls sit at PE row
        # groups 0-1 / 2-3 and run concurrently, and one exp() covers both.
        def emit_attention(pr, interleave_proj=False, xt4s=None):
            hA, hB = 2 * pr, 2 * pr + 1
            for i4 in range(4):
                i0 = i4 * 512
                po_A = ps_o.tile([65, 512], F32, tag="po", name=f"poA_{pr}_{i4}")
                po_B = ps_o.tile([65, 512], F32, tag="po", name=f"poB_{pr}_{i4}")
                for j in range(NO):
                    kt = k_sb[pr][j // 4]
                    jo = (j % 4) * P
                    qt = q_sb[pr][i4]
                    stm = ps_st.tile([P, 1024], F32, tag="st", name=f"st_{j}")
                    nc.tensor.matmul(
                        stm[:, 0:512],
                        _d(kt[0:64, jo : jo + P]),
                        _d(qt[0:64, :]),
                        start=True,
                        stop=True,
                    )
                    nc.tensor.matmul(
                        stm[:, 512:1024],
                        _d(kt[64:128, jo : jo + P]),
                        _d(qt[64:128, :]),
                        start=True,
                        stop=True,
                    )
                    ptile = pt_pool.tile([P, 1024], F32, tag="pt", name=f"pt_{j}")
                    nc.scalar.activation(
                        out=_r(ptile),
                        in_=stm,
                        func=mybir.ActivationFunctionType.Exp,
                        scale=SCALE,
                    )
                    if xt4s is not None and i4 == 0:
                        # produce v[j] just before its first consumer, reusing
                        # the x slices already in SBUF from the q/k pass; these
                        # matmuls fill PE gaps while the scalar engine exps
                        emit_v_chunk(j, xt4s[j // 4])
                    nc.tensor.matmul(
                        po_A,
                        _d(v_sb[j][:, hA, :]),
                        _d(ptile[:, 0:512]),
                        start=(j == 0),
                        stop=(j == NO - 1),
                    )
                    nc.tensor.matmul(
                        po_B,
                        _d(v_sb[j][:, hB, :]),
                        _d(ptile[:, 512:1024]),
                        start=(j == 0),
                        stop=(j == NO - 1),
                    )
                emit_normalize(po_A, pr, 0, i0)
                emit_normalize(po_B, pr, 64, i0)
                if interleave_proj:
                    emit_proj(pr, no_range=range(4 * i4, 4 * i4 + 4))

        def emit_proj(pr, no_range=None):
            # per-pair projection partial: out3[pr] = ot_pair.T @ wp[pr]
            # (the host sums the three pair partials; this removes the
            # cross-pair barrier and overlaps proj with the next pair)
            for no in (no_range if no_range is not None else range(NO)):
                o_sb = outp.tile([P, C], F32, tag="o", name=f"o_{pr}_{no}")
                for ob, width in ((0, 512), (1, 256)):
                    pp = ps1.tile([P, 512], F32, tag="pqk", name=f"pp_{pr}_{no}_{ob}")
                    nc.tensor.matmul(
                        pp[:, 0:width],
                        _d(ot_sb[pr][:, no * P : (no + 1) * P]),
                        _d(wp_sb[:, pr, ob * 512 : ob * 512 + width]),
                        start=True,
                        stop=True,
                    )
                    nc.vector.tensor_copy(
                        out=o_sb[:, ob * 512 : ob * 512 + width], in_=pp[:, 0:width]
                    )
                nc.sync.dma_start(
                    out=out3[pr, no * P : (no + 1) * P, :], in_=o_sb
                )

        # emission order = scheduling priority. Minimal weights first so
        # compute starts ~10us in; qkv pass pr runs in PE slack during
        # attention pr-1; proj pr-1 runs during attention pr; the last
        # pair's proj interleaves into its own attention blocks.
        load_wq([(0, P), (F, F + P), (2 * F, 3 * F)])  # q0, k0, v
        xt4s0 = emit_qkv_pass(0)
        emit_attention(0, xt4s=xt4s0)
        load_wq([(P, F), (F + P, 2 * F)])              # q1/q2, k1/k2
        for fo in range(FO):
            nc.sync.dma_start(
                out=_r(wp_sb[:, fo, :]),
                in_=_r(wprojT[fo * P : (fo + 1) * P, :]),
            )
        for pr in range(1, FO):
            emit_qkv_pass(pr)
            emit_proj(pr - 1)
            emit_attention(pr, interleave_proj=(pr == FO - 1))


_NC_CACHE = {}


def build_bass():
    key = _MM_DT_NAME
    if key in _NC_CACHE:
        return _NC_CACHE[key]
    nc = bass.Bass("TRN2")
    with tile.TileContext(nc) as tc:
        with ExitStack() as ctx:
            _emit(nc, tc, ctx)
    _split_multiwaits(nc)
    _NC_CACHE[key] = nc
    return nc


def make_in_maps(x, w_qkv, w_proj):
    x = np.asarray(x, dtype=np.float32)
    w_qkv = np.asarray(w_qkv, dtype=np.float32)
    w_proj = np.asarray(w_proj, dtype=np.float32)
    wq, wk, wv = w_qkv[0:C], w_qkv[C : 2 * C], w_qkv[2 * C : 3 * C]
    in_maps = []
    for c in range(NCORES):
        b, g = divmod(c, 2)
        sl = slice(g * F, (g + 1) * F)
        wslice = np.concatenate([wq[sl], wk[sl], wv[sl]], axis=0)  # [1152, 768]
        in_maps.append(
            {
                "xT": np.ascontiguousarray(x[b].T),
                "wqkvT": np.ascontiguousarray(wslice.T),
                "wprojT": np.ascontiguousarray(w_proj[:, sl].T),
            }
        )
    return in_maps


def gather_output(parts, b_proj):
    """parts: 8 arrays [FO, N, C] (pair partials per core)."""
    outv = np.empty((B, N, C), np.float32)
    for b in range(B):
        outv[b] = parts[2 * b].sum(axis=0) + parts[2 * b + 1].sum(axis=0)
    outv += np.asarray(b_proj, dtype=np.float32)[None, None, :]
    return outv


def kernel(x, w_qkv, w_proj, b_proj, _run_kwargs=None):
    nc = build_bass()
    in_maps = make_in_maps(x, w_qkv, w_proj)
    res = bass_utils.run_bass_kernel_spmd(
        nc, in_maps, core_ids=list(range(NCORES)), **(_run_kwargs or {})
    )
    parts = [r["out3"] for r in res.results]
    outv = gather_output(parts, b_proj)
    if _run_kwargs is not None:
        kernel.last_results = res
    return outv


# revision 34
# speedup vs baseline: 1.0101x; 1.0101x over previous
"""Trainium2 Bass kernel for nn_Attention (B=4, N=2048, C=768, H=12).

Sharding: 8 cores = 4 batches x 2 head-groups (6 heads each), Megatron-style
tensor parallel on the heads. Each core computes qkv for its head slice,
attention for 6 heads, and per-head-pair output-projection partials
out3 [3, 2048, 768]. The host sums the 3 pair partials of the 2 cores
covering each batch and adds the bias.

Per-core attention scheme (no transposes anywhere):
  - q,k stored [d, n] (feature-major) straight out of the QKV matmul; heads
    packed in pairs per 128-partition group (head 2p -> partitions 0-63,
    head 2p+1 -> 64-127).
  - S^T tiles [128 j, i] = k_chunk.T @ q  (K=64 matmul). exp() on scalar
    engine reads PSUM, writes SBUF. No max subtraction (logits are O(10);
    softmax is shift-invariant so this only perturbs rounding).
  - v stored [n, d] with an extra ones column; PV matmul lhsT=v[j,0:65],
    rhs=exp(S^T) accumulates [65, 512] where row 64 = sum_j exp = Z.
  - normalize: 1/Z broadcast across partitions via a DRAM-bounce DMA, one
    DVE multiply; odd heads' results are DMA'd up to partitions 64-127
    (engines cannot shift partitions; DMA can).

Matmuls default to float32r (TF32-class, 4x faster than fp32 on the PE;
measured 3.8e-4 scale-relative absmax error vs the fp32 reference).
Set KERNEL_MM_DT=float32 for full fp32 precision (3.4e-6) at ~3x the time.
"""

import os
import sys
from contextlib import ExitStack

if "/opt/trn_rl_repo" not in sys.path:
    sys.path.insert(0, "/opt/trn_rl_repo")

import numpy as np

import concourse.bass as bass
import concourse.mybir as mybir
import concourse.tile as tile
from concourse import bass_utils

F32 = mybir.dt.float32

B, N, C = 4, 2048, 768
NH, D = 12, 64
SCALE = D ** -0.5
HPC = NH // 2          # heads per core
F = HPC * D            # 384 per-core features per projection
QKVF = 3 * F           # 1152
P = 128
CO = C // P            # 6 contraction chunks
FO = F // P            # 3 feature chunks (head pairs)
NO = N // P            # 16 token chunks of 128
NCORES = 8

_MM_DT_NAME = os.environ.get("KERNEL_MM_DT", "float32r")
MM_DT = getattr(mybir.dt, _MM_DT_NAME)


def _d(ap):
    """Cast an fp32 AP to the matmul compute dtype (bitcast, same bytes)."""
    return ap.bitcast(MM_DT) if MM_DT != F32 else ap


def _r(ap):
    """Cast a producer OUT AP feeding a matmul to the compute dtype, so the
    producing engine rounds to fp32r (walrus verifies this chain)."""
    return ap.bitcast(MM_DT) if MM_DT == mybir.dt.float32r else ap


def _split_multiwaits(nc):
    """This container's walrus accepts at most ONE sync-wait per instruction.

    Split any instruction carrying N>1 waits into (N-1) single-wait NOPs on
    the same engine queue placed immediately before it (engine queues are
    FIFO, so the semantics are identical)."""
    ctr = 0
    for f in nc.m.functions:
        for blk in f.blocks:
            insts = blk.instructions
            out = []
            changed = False
            for ins in insts:
                si = ins.sync_info
                if si is not None and len(si.on_wait) > 1:
                    changed = True
                    waits = list(si.on_wait)
                    for ww in waits[:-1]:
                        nop = mybir.InstNoOp(name=f"zzsplitw_{ctr}", ins=[], outs=[])
                        ctr += 1
                        nop.engine = ins.engine
                        nop.sync_info = mybir.SyncInfo(on_wait=[ww], on_update=[])
                        out.append(nop)
                    ins.sync_info = mybir.SyncInfo(
                        on_wait=[waits[-1]], on_update=list(si.on_update)
                    )
                out.append(ins)
            if changed:
                blk.instructions = out
    return nc


def _emit(nc, tc, ctx):
    xT = nc.dram_tensor("xT", [C, N], F32, kind="ExternalInput").ap()
    # five contiguous weight sections (fully linear DMA reads; a single
    # [C, 1152] tensor would make every section load a 512B-strided gather
    # during the bandwidth-bound lead-in)
    wq_secs = {
        lo: nc.dram_tensor(f"wq{lo}", [C, hi - lo], F32, kind="ExternalInput").ap()
        for lo, hi in ((0, P), (F, F + P), (2 * F, 3 * F), (P, F), (F + P, 2 * F))
    }
    wprojT = nc.dram_tensor("wprojT", [F, C], F32, kind="ExternalInput").ap()
    out3 = nc.dram_tensor("out3", [FO, N, C], F32, kind="ExternalOutput").ap()

    persist = ctx.enter_context(tc.tile_pool(name="persist", bufs=1))

    # q/k in [feature, token] layout, split per (pair, 512-token chunk) so
    # consumers wait only on the producer they actually need (Tile tracks
    # dependencies at whole-tile granularity).
    q_sb = [[persist.tile([P, 512], F32, tag=f"q{fo}_{n4}", name=f"q{fo}_{n4}")
             for n4 in range(4)] for fo in range(FO)]
    k_sb = [[persist.tile([P, 512], F32, tag=f"k{fo}_{n4}", name=f"k{fo}_{n4}")
             for n4 in range(4)] for fo in range(FO)]
    # v in [token, feature] layout per 128-token chunk, +1 ones column.
    v_sb = [persist.tile([P, HPC, D + 1], F32, tag=f"v{no}", name=f"v{no}") for no in range(NO)]
    # attention output per pair, [feature, token] layout; 2 rotating slots
    # (pair 2 reuses pair 0's slot once proj-0 has drained it)
    otp = ctx.enter_context(tc.tile_pool(name="otp", bufs=2))
    ot_sb = [otp.tile([P, N], F32, tag="ot", name=f"ot{pr}") for pr in range(FO)]
    wp_sb = persist.tile([P, FO, C], F32, tag="wp")

    ones_sb = persist.tile([P, HPC], F32, tag="ones")
    nc.vector.memset(ones_sb, 1.0)
    for no in range(NO):
        # DVE copy (not memset) so the output can be declared fp32r
        nc.vector.tensor_copy(out=_r(v_sb[no][:, :, D : D + 1]), in_=ones_sb)
    # dummy exp: pulls the ~2.7us ACT table load into the DMA lead-in window
    expwarm = persist.tile([P, HPC], F32, tag="expwarm")
    nc.scalar.activation(
        out=expwarm,
        in_=ones_sb,
        func=mybir.ActivationFunctionType.Exp,
        scale=1.0,
    )

    with (
        tc.tile_pool(name="wqp", bufs=1) as wqp,
        tc.tile_pool(name="xs", bufs=3) as xs_pool,
        tc.tile_pool(name="ptp", bufs=3) as pt_pool,
        tc.tile_pool(name="rp", bufs=2) as r_pool,
        tc.tile_pool(name="outp", bufs=1) as outp,
        tc.tile_pool(name="rd", bufs=3, space="DRAM") as rd_pool,
        tc.tile_pool(name="ps1", bufs=2, space="PSUM") as ps1,
        tc.tile_pool(name="ps_st", bufs=2, space="PSUM") as ps_st,
        tc.tile_pool(name="ps_o", bufs=2, space="PSUM") as ps_o,
    ):
        # weight tiles per (column-section, contraction chunk) so each qkv
        # matmul depends on exactly one DMA
        wq_tiles = {}

        def load_wq(slices):
            for lo, hi in slices:
                for co in range(CO):
                    t = wqp.tile([P, hi - lo], F32, tag=f"wq_{lo}_{co}",
                                 name=f"wq_{lo}_{co}")
                    wq_tiles[(lo, co)] = t
                    nc.sync.dma_start(
                        out=_r(t),
                        in_=_r(wq_secs[lo][co * P : (co + 1) * P, :]),
                    )

        def wq_slice(foff, co, width=P):
            """AP for weight columns [foff, foff+width) of chunk co."""
            for lo, hi in ((0, P), (F, F + P), (2 * F, 3 * F), (P, F), (F + P, 2 * F)):
                if lo <= foff and foff + width <= hi:
                    return wq_tiles[(lo, co)][:, foff - lo : foff - lo + width]
            raise KeyError(foff)

        def emit_qkv_pass(fo):
            """q/k chunk fo over all tokens.

            Streams xT per 512-token slice (xT is re-read from DRAM once per
            pass; DMA is far from the bottleneck and this keeps SBUF free)."""
            xt4s = []
            for n4 in range(4):
                xt4 = xs_pool.tile([P, CO, 512], F32, tag="xt4",
                                   name=f"xt4_{fo}_{n4}")
                xt4s.append(xt4)
                # one DMA per contraction chunk so the first matmul can
                # start after ~256KB instead of the full 1.5MB slice
                for co in range(CO):
                    nc.sync.dma_start(
                        out=_r(xt4[:, co, :]),
                        in_=_r(xT[co * P : (co + 1) * P,
                                  n4 * 512 : (n4 + 1) * 512]),
                    )
                for dst, foff in ((q_sb[fo][n4], fo * P), (k_sb[fo][n4], F + fo * P)):
                    pq = ps1.tile([P, 512], F32, tag="pqk")
                    for co in range(CO):
                        nc.tensor.matmul(
                            pq,
                            _d(wq_slice(foff, co)),
                            _d(xt4[:, co, :]),
                            start=(co == 0),
                            stop=(co == CO - 1),
                        )
                    nc.vector.tensor_copy(out=_r(dst), in_=pq)
            return xt4s

        def emit_v_chunk(no, xtv):
            """v for one 128-token chunk, reading an [P, CO, 512] x-slice."""
            pv = ps1.tile([P, F], F32, tag="pqk", name=f"pv_{no}")
            for co in range(CO):
                nc.tensor.matmul(
                    pv,
                    _d(xtv[:, co, (no % 4) * P : (no % 4 + 1) * P]),
                    _d(wq_slice(2 * F, co, F)),
                    start=(co == 0),
                    stop=(co == CO - 1),
                )
            nc.vector.tensor_copy(
                out=_r(v_sb[no][:, :, 0:D]),
                in_=pv.rearrange("p (h d) -> p h d", h=HPC),
            )

        def emit_normalize(po, pr, plo, i512):
            # evacuate PSUM -> SBUF at once so the po slot frees for the next
            # i512 block (the normalize chain below has DMA latency in it)
            ov = r_pool.tile([65, 512], F32, tag="ov", name=f"ov_{pr}_{plo}_{i512}")
            # 1/Z lives on partition 64 (engines cannot move data across
            # partitions, so compute in place on lane 64); reading po directly
            # lets the broadcast DMA start before the row evacuation finishes
            nc.vector.reciprocal(out=ov[64:65, :], in_=po[64:65, :])
            nc.vector.tensor_copy(out=ov[0:64, :], in_=po[0:64, :])
            # partition-broadcast 1/Z: SBUF zero-step partition APs are
            # illegal, so bounce through DRAM (DRAM APs broadcast fine)
            rdram = rd_pool.tile([1, 512], F32, tag="rd", name=f"rd_{pr}_{plo}_{i512}")
            nc.sync.dma_start(out=rdram, in_=ov[64:65, :])
            rb = r_pool.tile([64, 512], F32, tag="rb", name=f"rb_{pr}_{plo}_{i512}")
            nc.sync.dma_start(out=rb, in_=rdram.to_broadcast([64, 512]))
            if plo == 0:
                nc.vector.tensor_mul(
                    out=_r(ot_sb[pr][0:64, i512 : i512 + 512]),
                    in0=ov[0:64, :],
                    in1=rb,
                )
            else:
                # odd head: normalize at partitions 0-63, then DMA up to
                # partitions 64-127 of ot
                nt = r_pool.tile([64, 512], F32, tag="nt", name=f"nt_{pr}_{i512}")
                nc.vector.tensor_mul(out=_r(nt), in0=ov[0:64, :], in1=rb)
                nc.sync.dma_start(
                    out=_r(ot_sb[pr][64:128, i512 : i512 + 512]), in_=_r(nt)
                )

        # interleave: qkv pass for a head pair, then that pair's attention.
        # Both heads of a pair share one [128, 1024] S^T tile (head A cols
        # 0-511, head B cols 512-1023): their K=64 matmuls sit at PE row
        # groups 0-1 / 2-3 and run concurrently, and one exp() covers both.
        def emit_attention(pr, interleave_proj=False, xt4s=None):
            hA, hB = 2 * pr, 2 * pr + 1
            for i4 in range(4):
                i0 = i4 * 512
                po_A = ps_o.tile([65, 512], F32, tag="po", name=f"poA_{pr}_{i4}")
                po_B = ps_o.tile([65, 512], F32, tag="po", name=f"poB_{pr}_{i4}")
                for j in range(NO):
                    kt = k_sb[pr][j // 4]
                    jo = (j % 4) * P
                    qt = q_sb[pr][i4]
                    stm = ps_st.tile([P, 1024], F32, tag="st", name=f"st_{j}")
                    nc.tensor.matmul(
                        stm[:, 0:512],
                        _d(kt[0:64, jo : jo + P]),
                        _d(qt[0:64, :]),
                        start=True,
                        stop=True,
                    )
                    nc.tensor.matmul(
                        stm[:, 512:1024],
                        _d(kt[64:128, jo : jo + P]),
                        _d(qt[64:128, :]),
                        start=True,
                        stop=True,
                    )
                    ptile = pt_pool.tile([P, 1024], F32, tag="pt", name=f"pt_{j}")
                    nc.scalar.activation(
                        out=_r(ptile),
                        in_=stm,
                        func=mybir.ActivationFunctionType.Exp,
                        scale=SCALE,
                    )
                    if xt4s is not None and i4 == 0:
                        # produce v[j] just before its first consumer, reusing
                        # the x slices already in SBUF from the q/k pass; these
                        # matmuls fill PE gaps while the scalar engine exps
                        emit_v_chunk(j, xt4s[j // 4])
                    nc.tensor.matmul(
                        po_A,
                        _d(v_sb[j][:, hA, :]),
                        _d(ptile[:, 0:512]),
                        start=(j == 0),
                        stop=(j == NO - 1),
                    )
                    nc.tensor.matmul(
                        po_B,
                        _d(v_sb[j][:, hB, :]),
                        _d(ptile[:, 512:1024]),
                        start=(j == 0),
                        stop=(j == NO - 1),
                    )
                emit_normalize(po_A, pr, 0, i0)
                emit_normalize(po_B, pr, 64, i0)
                if interleave_proj:
                    emit_proj(pr, no_range=range(4 * i4, 4 * i4 + 4))

        def emit_proj(pr, no_range=None):
            # per-pair projection partial: out3[pr] = ot_pair.T @ wp[pr]
            # (the host sums the three pair partials; this removes the
            # cross-pair barrier and overlaps proj with the next pair)
            for no in (no_range if no_range is not None else range(NO)):
                o_sb = outp.tile([P, C], F32, tag="o", name=f"o_{pr}_{no}")
                for ob, width in ((0, 512), (1, 256)):
                    pp = ps1.tile([P, 512], F32, tag="pqk", name=f"pp_{pr}_{no}_{ob}")
                    nc.tensor.matmul(
                        pp[:, 0:width],
                        _d(ot_sb[pr][:, no * P : (no + 1) * P]),
                        _d(wp_sb[:, pr, ob * 512 : ob * 512 + width]),
                        start=True,
                        stop=True,
                    )
                    nc.vector.tensor_copy(
                        out=o_sb[:, ob * 512 : ob * 512 + width], in_=pp[:, 0:width]
                    )
                nc.sync.dma_start(
                    out=out3[pr, no * P : (no + 1) * P, :], in_=o_sb
                )

        # emission order = scheduling priority. Minimal weights first so
        # compute starts ~10us in; qkv pass pr runs in PE slack during
        # attention pr-1; proj pr-1 runs during attention pr; the last
        # pair's proj interleaves into its own attention blocks.
        load_wq([(0, P), (F, F + P), (2 * F, 3 * F)])  # q0, k0, v
        xt4s0 = emit_qkv_pass(0)
        emit_attention(0, xt4s=xt4s0)
        load_wq([(P, F), (F + P, 2 * F)])              # q1/q2, k1/k2
        for fo in range(FO):
            nc.sync.dma_start(
                out=_r(wp_sb[:, fo, :]),
                in_=_r(wprojT[fo * P : (fo + 1) * P, :]),
            )
        for pr in range(1, FO):
            emit_qkv_pass(pr)
            emit_proj(pr - 1)
            emit_attention(pr, interleave_proj=(pr == FO - 1))


_NC_CACHE = {}


def build_bass():
    key = _MM_DT_NAME
    if key in _NC_CACHE:
        return _NC_CACHE[key]
    nc = bass.Bass("TRN2")
    with tile.TileContext(nc) as tc:
        with ExitStack() as ctx:
            _emit(nc, tc, ctx)
    _split_multiwaits(nc)
    _NC_CACHE[key] = nc
    return nc


def make_in_maps(x, w_qkv, w_proj):
    x = np.asarray(x, dtype=np.float32)
    w_qkv = np.asarray(w_qkv, dtype=np.float32)
    w_proj = np.asarray(w_proj, dtype=np.float32)
    wq, wk, wv = w_qkv[0:C], w_qkv[C : 2 * C], w_qkv[2 * C : 3 * C]
    in_maps = []
    for c in range(NCORES):
        b, g = divmod(c, 2)
        sl = slice(g * F, (g + 1) * F)
        wslice = np.concatenate([wq[sl], wk[sl], wv[sl]], axis=0)  # [1152, 768]
        wT = np.ascontiguousarray(wslice.T)  # [768, 1152]
        m = {
            "xT": np.ascontiguousarray(x[b].T),
            "wprojT": np.ascontiguousarray(w_proj[:, sl].T),
        }
        for lo, hi in ((0, 128), (384, 512), (768, 1152), (128, 384), (512, 768)):
            m[f"wq{lo}"] = np.ascontiguousarray(wT[:, lo:hi])
        in_maps.append(m)
    return in_maps


def gather_output(parts, b_proj):
    """parts: 8 arrays [FO, N, C] (pair partials per core)."""
    outv = np.empty((B, N, C), np.float32)
    for b in range(B):
        outv[b] = parts[2 * b].sum(axis=0) + parts[2 * b + 1].sum(axis=0)
    outv += np.asarray(b_proj, dtype=np.float32)[None, None, :]
    return outv


def kernel(x, w_qkv, w_proj, b_proj, _run_kwargs=None):
    nc = build_bass()
    in_maps = make_in_maps(x, w_qkv, w_proj)
    res = bass_utils.run_bass_kernel_spmd(
        nc, in_maps, core_ids=list(range(NCORES)), **(_run_kwargs or {})
    )
    parts = [r["out3"] for r in res.results]
    outv = gather_output(parts, b_proj)
    if _run_kwargs is not None:
        kernel.last_results = res
    return outv
